# revision 12
# baseline (speedup 1.0000x reference)
"""Trainium2 Bass kernel for nn_DynamicWeightAttention.

Reference computation (per token t = (bt, n, h)):
    fused = concat(dyn[bt,n,h,:], static[n,h,:])            # C=32
    normed = LayerNorm(fused; gamma, beta, eps=1e-4)
    hmid   = tanh(normed @ w1 + b1)                         # HID=64
    score  = hmid @ w2 + b2                                 # scalar
    out[bt,n,:] = softmax over h of score                   # H=16

Strategy (8 NeuronCores, data-sharded over N: core c owns n in [32c, 32c+32)):
  - LayerNorm affine + static features fold host-side into per-n block
    weight matrices (static contribution enters via selector rows), so
    only the 16 dynamic features stream from HBM (pre-cast to bf16 and
    pre-swizzled host-side so each DMA is contiguous 8KB runs).
  - Per core, partition p owns bt-octet [8p, 8p+8); chunk e selects one
    bt per partition. dyn lands directly in a [128, 32n, 4v, 4a, 16f]
    slab (h = 4v + a); aux rows (inv selector diag / mean*inv / const 1)
    live in a separate [128, 32n, 4v, 32] slab.
  - Stats (mean/var) on DVE via bf16 pairwise trees (square on ACT, same
    activation table as tanh/exp so no table reloads); invstd via
    bit-trick rsqrt + 2 Newton steps; dyn scaled by invstd in-place on
    GPSIMD.
  - Per half-chunk, two xbar transposes produce K-major [96, 16n, 4v,
    128p] tiles; mm1: 2 bf16 K=96 matmuls per n -> h_pre for the 2048
    tokens of that n in PSUM; tanh on ACT into a full-half ht buffer;
    mm2: s-outer loop of 8 stationaries x 4 nb accumulates scores into
    one [128=(nb,h), 512=(j,c)] PSUM tile per chunk.
  - Softmax per chunk: ONE exp (ACT) -> ONE blockdiag-ones matmul for
    denominators (PE) -> reciprocal_approx_fast (DVE) -> multiply (DVE)
    -> one xbar transpose + ACT permute-copy to token-major f32 -> one
    output DMA.
  - Emission is software-pipelined: chunk c+1's stats phase is emitted
    before chunk c's matmul phase so per-engine in-order streams overlap.
"""
import os

import numpy as np
import ml_dtypes

import concourse.bacc as bacc
import concourse.mybir as mybir
from concourse.ap import AP as BassAP
from concourse.tile import TileContext
from concourse.bass_utils import run_bass_kernel_spmd

F32 = mybir.dt.float32
BF16 = mybir.dt.bfloat16
U32 = mybir.dt.uint32
AT = mybir.AluOpType
AFT = mybir.ActivationFunctionType

B_T, N, H, PD, PS, HID = 1024, 256, 16, 16, 16, 64
NCORES = 8
NC_N = N // NCORES          # 32 n's per core
EPS = 1e-4
MAGIC = float(0x5F3759DF)
KROWS = 128                 # 64 dyn + 32 aux + 32 pad K rows (xbar needs 128-col groups)

_cached = {}


def _host_prep(dynamic_features, static_features, ln_gamma, ln_beta, w1, b1, w2, b2):
    """Fold LN affine + static features into per-core packed weights."""
    g = np.asarray(ln_gamma, np.float32)
    be = np.asarray(ln_beta, np.float32)
    w1 = np.asarray(w1, np.float32)
    b1 = np.asarray(b1, np.float32)
    w2v = np.asarray(w2, np.float32).reshape(HID)
    st = np.asarray(static_features, np.float32)

    w1g = w1 * g[:, None]                      # [32, 64]
    w1d = w1g[:PD]                             # [16, 64] dyn part
    w1s = w1g[PD:]                             # [16, 64] static part
    cw = w1g.sum(0)                            # [64]
    b1p = b1 + be @ w1                         # [64]

    m2 = np.zeros((8, 128, 16), np.float32)
    for v in range(4):
        for half in range(2):
            m2[half * 4 + v, 0:64, 4 * v + 2 * half] = w2v
            m2[half * 4 + v, 64:128, 4 * v + 2 * half + 1] = w2v
    # quadrant-aligned block-diagonal ones [128, 128]: 16x16 blocks at the
    # four 32-aligned quadrant bases (denominator sums over the h
    # partitions within each quadrant's valid band)
    onesblk = np.zeros((128, 128), np.float32)
    for q in range(4):
        onesblk[32 * q:32 * q + 16, 32 * q:32 * q + 16] = 1.0
        # identity in the junk bands keeps their denominators finite (those
        # rows are never read downstream but must not produce NaN)
        for i in range(16):
            onesblk[32 * q + 16 + i, 32 * q + 16 + i] = 1.0

    dyn = np.asarray(dynamic_features, np.float32)

    per_core = []
    for c in range(NCORES):
        stc = st[c * NC_N:(c + 1) * NC_N]      # [32, 16, 16]
        sp = np.einsum("nhp,pd->nhd", stc, w1s)  # [32, 16, 64]
        s_st = stc.sum(-1)                     # [32, 16]
        q_st = (stc ** 2).sum(-1)              # [32, 16]

        # K-row layout (r in [0,128)):
        #   r in [0,64):  dyn, a = r//16, f = r%16
        #   r in [64,96): aux, a = (r-64)//8, w = (r-64)%8:
        #                 w in 0..3 -> sel row v'=w (value inv iff v'==v),
        #                 w=4 -> mean*inv, w=5 -> const 1, w in 6..7 -> pad
        #   r in [96,128): pad (zero weights, zero slab values)
        wa = np.zeros((NC_N, KROWS, 128), np.float32)
        wb = np.zeros((NC_N, KROWS, 128), np.float32)
        for n in range(NC_N):
            for a in range(4):
                tgt = wa if a < 2 else wb
                mcol = 64 * (a % 2)
                tgt[n, 16 * a:16 * a + 16, mcol:mcol + 64] = w1d
                for vv in range(4):
                    tgt[n, 64 + 8 * a + vv, mcol:mcol + 64] = sp[n, 4 * vv + a]
                tgt[n, 64 + 8 * a + 4, mcol:mcol + 64] = -cw
                tgt[n, 64 + 8 * a + 5, mcol:mcol + 64] = b1p

        # dyn swizzle: [bt, n, h, f] -> [p, e, n, v, a, f] with bt = 8p + e,
        # h = 4v + a, pre-cast to bf16
        dc = dyn[:, c * NC_N:(c + 1) * NC_N]   # [1024, 32, 16, 16]
        dswz = np.ascontiguousarray(
            dc.reshape(128, 8, NC_N, 4, 4, PD)).astype(ml_dtypes.bfloat16)

        per_core.append({
            "dyn": dswz,
            "wa": wa.astype(ml_dtypes.bfloat16),
            "wb": wb.astype(ml_dtypes.bfloat16),
            "m2": m2.astype(ml_dtypes.bfloat16),
            "onesblk": onesblk.astype(ml_dtypes.bfloat16),
            "sst32": np.ascontiguousarray((s_st / 32.0).reshape(1, 512).astype(np.float32)),
            "qst32": np.ascontiguousarray((q_st / 32.0 + EPS).reshape(1, 512).astype(np.float32)),
        })
    return per_core


def build_nc(n_chunks=8):
    nc = bacc.Bacc("TRN2", target_bir_lowering=False, debug=False, num_devices=NCORES)
    dyn = nc.dram_tensor("dyn", [128, 8, NC_N, 4, 4, PD], BF16, kind="ExternalInput")
    wa_d = nc.dram_tensor("wa", [NC_N, KROWS, 128], BF16, kind="ExternalInput")
    wb_d = nc.dram_tensor("wb", [NC_N, KROWS, 128], BF16, kind="ExternalInput")
    m2_d = nc.dram_tensor("m2", [8, 128, 16], BF16, kind="ExternalInput")
    ones_d = nc.dram_tensor("onesblk", [128, 128], BF16, kind="ExternalInput")
    sst_d = nc.dram_tensor("sst32", [1, 512], F32, kind="ExternalInput")
    qst_d = nc.dram_tensor("qst32", [1, 512], F32, kind="ExternalInput")
    out_d = nc.dram_tensor("out", [B_T, NC_N, H], F32, kind="ExternalOutput")

    NH = 16  # n's per half-chunk

    with TileContext(nc) as tc:
        with tc.tile_pool(name="const", bufs=1) as cpool, \
             tc.tile_pool(name="stats", bufs=1) as stpool, \
             tc.tile_pool(name="stg", bufs=2) as stgpool, \
             tc.tile_pool(name="x2p", bufs=1) as x2pool, \
             tc.tile_pool(name="invp", bufs=1) as invpool, \
             tc.tile_pool(name="tr", bufs=1) as trpool, \
             tc.tile_pool(name="hid", bufs=1) as hpool, \
             tc.tile_pool(name="sm", bufs=1) as smpool, \
             tc.tile_pool(name="ot", bufs=2) as otpool, \
             tc.tile_pool(name="ps1", bufs=2, space="PSUM") as ps1pool, \
             tc.tile_pool(name="ps2", bufs=1, space="PSUM") as ps2pool, \
             tc.tile_pool(name="psd", bufs=2, space="PSUM") as psdpool:

            # ---- constants / weights (loaded once) ----
            wat = cpool.tile([KROWS, NC_N, 128], BF16)
            nc.sync.dma_start(wat[:, :, :], wa_d[:, :, :].rearrange("n k m -> k n m"))
            wbt = cpool.tile([KROWS, NC_N, 128], BF16)
            nc.sync.dma_start(wbt[:, :, :], wb_d[:, :, :].rearrange("n k m -> k n m"))
            m2t = cpool.tile([128, 8, 16], BF16)
            nc.sync.dma_start(m2t[:, :, :], m2_d[:, :, :].rearrange("s k m -> k s m"))
            onest = cpool.tile([128, 128], BF16)
            nc.sync.dma_start(onest[:, :], ones_d[:, :])
            sstt = cpool.tile([128, 512], F32)
            nc.sync.dma_start(sstt[0:1, :], sst_d[:, :])
            nc.gpsimd.partition_broadcast(sstt[:, :], sstt[0:1, :], channels=128)
            qstt = cpool.tile([128, 512], F32)
            nc.sync.dma_start(qstt[0:1, :], qst_d[:, :])
            nc.gpsimd.partition_broadcast(qstt[:, :], qstt[0:1, :], channels=128)

            # ---- persistent slabs (2, rotated per chunk) ----
            # slab cols: [0:64) dyn (a,f), [64:96) aux, [96:128) pad
            dslabs = []
            for i in range(2):
                ds = cpool.tile([128, NC_N, 4, 128], BF16, tag=f"dslab{i}")
                nc.vector.memset(ds[:, :, :, 64:128], 0.0)
                dsf = ds[:, :, :, :].rearrange("p n v c -> p (n v c)")
                p0 = list(dsf.ap)[0]
                # const-1 rows at col 64 + 8a + 5
                nc.vector.memset(BassAP(dsf.tensor, dsf.offset + 69,
                                        [p0, [512, NC_N], [128, 4], [8, 4]]), 1.0)
                dslabs.append(ds)

            def stats_phase(b8):
                """Load chunk b8, compute invstd, fill aux + scale dyn."""
                ds = dslabs[b8 % 2]
                dsf = ds[:, :, :, :].rearrange("p n v c -> p (n v c)")
                p0 = list(dsf.ap)[0]
                ssum = stpool.tile([128, 512], F32, tag="ssum")
                q = stpool.tile([128, 512], F32, tag="q")
                stgs = []
                for hc in range(2):
                    n0 = hc * NH
                    stg = stgpool.tile([128, NH, 4, 4, PD], BF16, tag="stg")
                    stgs.append(stg)
                    nc.gpsimd.dma_start(stg[:, :, :, :, :], dyn[:, b8, n0:n0 + NH, :, :, :])
                    dh = stg[:, :, :, :, :].rearrange("p n v g f -> p (n v g) f")

                    # sum tree (bf16, 2x mode)
                    t8 = stpool.tile([128, 256, 8], BF16, tag="t8")
                    nc.vector.tensor_tensor(t8[:, :, :], dh[:, :, 0:8], dh[:, :, 8:16], AT.add)
                    t4 = stpool.tile([128, 256, 4], BF16, tag="t4")
                    nc.vector.tensor_tensor(t4[:, :, :], t8[:, :, 0:4], t8[:, :, 4:8], AT.add)
                    t2 = stpool.tile([128, 256, 2], BF16, tag="t2")
                    nc.vector.tensor_tensor(t2[:, :, :], t4[:, :, 0:2], t4[:, :, 2:4], AT.add)
                    nc.vector.tensor_tensor(ssum[:, n0 * 16:(n0 + NH) * 16],
                                            t2[:, :, 0], t2[:, :, 1], AT.add)

                    # square on ACT (same act table as tanh/exp), then tree
                    # (q-tree reuses the t-tree buffers)
                    x2 = x2pool.tile([128, 256, PD], BF16, tag="x2")
                    nc.scalar.activation(x2[:, :, :], dh, AFT.Square)
                    q8 = stpool.tile([128, 256, 8], BF16, tag="t8")
                    nc.vector.tensor_tensor(q8[:, :, :], x2[:, :, 0:8], x2[:, :, 8:16], AT.add)
                    q4 = stpool.tile([128, 256, 4], BF16, tag="t4")
                    nc.vector.tensor_tensor(q4[:, :, :], q8[:, :, 0:4], q8[:, :, 4:8], AT.add)
                    q2 = stpool.tile([128, 256, 2], BF16, tag="t2")
                    nc.vector.tensor_tensor(q2[:, :, :], q4[:, :, 0:2], q4[:, :, 2:4], AT.add)
                    nc.vector.tensor_tensor(q[:, n0 * 16:(n0 + NH) * 16],
                                            q2[:, :, 0], q2[:, :, 1], AT.add)

                # full-chunk stats chain [128, 512]
                mean = stpool.tile([128, 512], F32, tag="mean")
                nc.vector.scalar_tensor_tensor(mean[:, :], ssum[:, :], 1.0 / 32, sstt[:, :], AT.mult, AT.add)
                vareps = stpool.tile([128, 512], F32, tag="vareps")
                nc.vector.scalar_tensor_tensor(vareps[:, :], q[:, :], 1.0 / 32, qstt[:, :], AT.mult, AT.add)
                m2neg = stpool.tile([128, 512], F32, tag="m2neg")
                nc.vector.scalar_tensor_tensor(m2neg[:, :], mean[:, :], -1.0, mean[:, :], AT.mult, AT.mult)
                nc.vector.tensor_tensor(vareps[:, :], vareps[:, :], m2neg[:, :], AT.add)

                # rsqrt: bit-trick seed + 2 Newton steps
                seed = stpool.tile([128, 512], U32, tag="q")
                nc.vector.tensor_scalar(seed[:, :], vareps[:, :].bitcast(U32), 1, None, AT.logical_shift_right)
                nc.vector.tensor_scalar(seed[:, :], seed[:, :], -1.0, MAGIC, AT.mult, AT.add)
                inv = invpool.tile([128, 512], F32, tag="inv")
                tmp = stpool.tile([128, 512], F32, tag="tmp")
                y0 = seed[:, :].bitcast(F32)
                nc.vector.tensor_tensor(tmp[:, :], y0, y0, AT.mult)
                nc.vector.scalar_tensor_tensor(tmp[:, :], tmp[:, :], -0.5, vareps[:, :], AT.mult, AT.mult)
                nc.vector.tensor_scalar(tmp[:, :], tmp[:, :], 1.5, None, AT.add)
                nc.vector.tensor_tensor(inv[:, :], y0, tmp[:, :], AT.mult)
                nc.vector.tensor_tensor(tmp[:, :], inv[:, :], inv[:, :], AT.mult)
                nc.vector.scalar_tensor_tensor(tmp[:, :], tmp[:, :], -0.5, vareps[:, :], AT.mult, AT.mult)
                nc.vector.tensor_scalar(tmp[:, :], tmp[:, :], 1.5, None, AT.add)
                nc.vector.tensor_tensor(inv[:, :], inv[:, :], tmp[:, :], AT.mult)

                minv = stpool.tile([128, 512], F32, tag="m2neg")
                nc.vector.tensor_tensor(minv[:, :], mean[:, :], inv[:, :], AT.mult)

                inv_nva = inv[:, :].rearrange("p (n v a) -> p n v a", n=NC_N, v=4)
                # aux diag: col 64 + 8a + v gets inv (others stay 0 from init)
                nc.vector.tensor_copy(
                    BassAP(dsf.tensor, dsf.offset + 64,
                           [p0, [512, NC_N], [129, 4], [8, 4]]), inv_nva)
                # mean*inv at col 64 + 8a + 4
                nc.vector.tensor_copy(
                    BassAP(dsf.tensor, dsf.offset + 68,
                           [p0, [512, NC_N], [128, 4], [8, 4]]),
                    minv[:, :].rearrange("p (n v a) -> p n v a", n=NC_N, v=4))

                # scale dyn by invstd: stg (contiguous read) -> slab (GPSIMD)
                for hc in range(2):
                    n0 = hc * NH
                    inv_h = (inv_nva[:, n0:n0 + NH, :, :]
                             .rearrange("p n v (a o) -> p n v a o", o=1)
                             .broadcast_to([128, NH, 4, 4, PD]))
                    dhv = ds[:, n0:n0 + NH, :, 0:64].rearrange(
                        "p n v (a f) -> p n v a f", a=4)
                    nc.gpsimd.tensor_tensor(dhv, stgs[hc][:, :, :, :, :], inv_h, AT.mult)

            # persistent score psum tiles: tile x holds nb = 2q + x at
            # partition base 32q (quadrant-aligned for the PE); partitions
            # 32q+16..32q+32 are junk bands, zeroed once here so exp of
            # them stays finite.
            scps2 = []
            for x in range(2):
                sc = ps2pool.tile([128, 4, 128], F32, tag=f"scps{x}")
                nc.vector.memset(sc[:, :, :], 0.0)
                scps2.append(sc)

            def mm_phase(b8):
                """Transpose chunk b8 halves, mm1/tanh/mm2, softmax, output."""
                ds = dslabs[b8 % 2]
                for hc in range(2):
                    n0 = hc * NH
                    trt = trpool.tile([KROWS, NH, 4, 128], BF16, tag="tr")
                    nc.sync.dma_start_transpose(
                        trt[:, :, :, :],
                        ds[:, n0:n0 + NH, :, :].rearrange("p n v c -> p (n v c)"))
                    ht4 = hpool.tile([128, NH, 8, 128], BF16, tag="h")
                    for ni in range(NH):
                        n = n0 + ni
                        ps = ps1pool.tile([128, 1024], F32, tag="ps1")
                        rhs = trt[:, ni, :, :].rearrange("k v p -> k (v p)")
                        nc.tensor.matmul(ps[:, 0:512], wat[:, n, :], rhs, start=True, stop=True)
                        nc.tensor.matmul(ps[:, 512:1024], wbt[:, n, :], rhs, start=True, stop=True)
                        nc.scalar.activation(
                            ht4[:, ni, :, :],
                            ps[:, :].rearrange("p (s c) -> p s c", s=8), AFT.Tanh)
                    # mm2: q-outer, s-mid, nb-pair inner — each m2 stationary
                    # serves the even/odd-nb pair (separate psum tiles, so
                    # only two accumulation groups are in flight and each
                    # lives in its own bank)
                    for ql in range(2):
                        q = 2 * hc + ql
                        for s in range(8):
                            for x in range(2):
                                nbl = 2 * ql + x
                                nc.tensor.matmul(
                                    scps2[x][32 * q:32 * q + 16, :, :],
                                    m2t[:, s, :],
                                    ht4[:, 4 * nbl:4 * nbl + 4, s, :],
                                    start=(s == 0), stop=(s == 7),
                                    tile_position=(0, 32 * q))

                # softmax over h (two instrs per stage per chunk)
                otc = otpool.tile([128, NC_N, H], F32, tag="otc")
                for x in range(2):
                    scv = scps2[x][:, :, :].rearrange("r j c -> r (j c)")
                    et = smpool.tile([128, 512], BF16, tag="e")
                    nc.scalar.activation(et[:, :], scv, AFT.Exp)
                    dps = psdpool.tile([128, 512], F32, tag="psd")
                    nc.tensor.matmul(dps[:, :], onest[:, :], et[:, :], start=True, stop=True)
                    rt = smpool.tile([128, 512], F32, tag="r")
                    nc.vector.reciprocal_approx_fast(rt[:, :], dps[:, :])
                    ft = smpool.tile([128, 512], BF16, tag="f")
                    nc.vector.tensor_tensor(ft[:, :], et[:, :], rt[:, :], AT.mult)

                    # [(q,t,h), (j,c)] -> [c, j, (q,t,h)] via xbar, then ACT
                    # permute-copy (skipping junk t=1 bands) to token-major
                    # f32: n = 8q + 4x + j
                    ott = otpool.tile([128, 4, 128], BF16, tag="ott")
                    nc.sync.dma_start_transpose(ott[:, :, :], ft[:, :])
                    ottv = ott[:, :, :].rearrange(
                        "c j (q t h) -> c q t j h", q=4, t=2)[:, :, 0, :, :]
                    otcv = otc[:, :, :].rearrange(
                        "c (q y j) h -> c q y j h", q=4, y=2)[:, :, x, :, :]
                    nc.scalar.activation(otcv, ottv, AFT.Copy)
                nc.gpsimd.dma_start(
                    out_d[:, :, :].rearrange("(p e) n h -> p e n h", e=8)[:, b8, :, :],
                    otc[:, :, :])

            # software pipeline, depth 1: stats(c+1) emitted before mm(c)
            stats_phase(0)
            for b8 in range(n_chunks):
                if b8 + 1 < n_chunks:
                    stats_phase(b8 + 1)
                mm_phase(b8)
    nc.compile()
    return nc


def kernel(**inputs):
    per_core = _host_prep(**inputs)
    if "nc" not in _cached:
        _cached["nc"] = build_nc()
    nc = _cached["nc"]
    trace = bool(os.environ.get("DWA_TRACE"))
    res = run_bass_kernel_spmd(nc, per_core, core_ids=list(range(NCORES)), trace=trace)
    if trace:
        print("HW exec time:", res.exec_time_ns, "ns")
        kernel.last_result = res
    out = np.empty((B_T, N, H), np.float32)
    for c in range(NCORES):
        out[:, c * NC_N:(c + 1) * NC_N, :] = res.results[c]["out"]
    return out
